# revision 16
# baseline (speedup 1.0000x reference)
"""Distributed Trainium2 (Bass/Tile) kernel for the DQN-style GNN message-passing
module.

Full-input contract: ``kernel(**inputs)`` takes the unsharded inputs exactly as
produced by ``setup_inputs()`` and returns the full output (shape ``(1,)``).

Sharding strategy (1D node partition, per the hint):
  - graph [N, N] row-sharded across 8 cores -> [N/8, N] per core
  - x / u row-sharded; thetas replicated
  - per-iteration global reduction: the local per-core sum sl = u.sum(rows) is
    folded through theta2 BEFORE communicating (AllReduce is linear), giving a
    [1, 64] row that is AllGathered (floor ~4.6us vs AllReduce ~9.7us) and
    combined on-chip with a ones-vector matmul.  The [1, 64] single-partition
    staging layout makes the HBM bounce DMA a single 256B descriptor instead
    of 64x4B (whose completion receipts cost ~10us+).
  - final readout: only core 0's output is read, and v's row lives on core 0,
    so the last collective is the same [1,64] AllGather with theta6 folded in.

Math (mirrors the reference's exact relu identity):
  c[r, :] = s_abs[r] * A + s_sum[r] * B
    with s_abs[r] = sum_j |g[r, j]|, s_sum[r] = sum_j g[r, j],
         A = 0.5 * |theta4| @ theta3, B = 0.5 * theta4 @ theta3
  a[r, :] = xf[r] * theta1

  Per-core state tile US [64 + 1 + 2*NCH, R] stacks u^T over per-row scalars
  [xf; abs chunk sums; plain chunk sums].  With the host-built stationary
  M = [-theta2; theta1; A (x NCH); B (x NCH)], a single matmul computes
    pre^T = M^T @ US = (a + c - u @ theta2)^T
  and the iteration is  u' = ReluAct(pre, bias = zs)  where zs [64,1] is the
  ones-combine of the AllGathered t2-folded rows.
"""

from contextlib import ExitStack

import numpy as np

import concourse.bass as bass
import concourse.tile as tile
from concourse import bacc, mybir
from concourse.bass_utils import run_bass_kernel_spmd

F32 = mybir.dt.float32
BF16 = mybir.dt.bfloat16
AX = mybir.AxisListType
ALU = mybir.AluOpType
ACTF = mybir.ActivationFunctionType

N_CORES = 8
DIM = 64

_program_cache: dict = {}


def _chunking(C: int):
    CH = 2048 if C >= 2048 else C
    return CH, C // CH


def build_program(R: int, C: int, D: int, T: int, n_cores: int = N_CORES):
    """Build + compile the per-core SPMD Bass program.

    R: local rows (N / n_cores), C: row length (N), D: dim, T: iterations.
    """
    assert R % 128 == 0 and D == 64
    NB = R // 128
    CH, NCH = _chunking(C)
    KM = D + 1 + 2 * NCH  # stationary contraction size
    FB = 512  # matmul free-dim chunk (one PSUM bank of f32)

    nc = bacc.Bacc(
        "TRN2",
        target_bir_lowering=False,
        debug=False,
        enable_asserts=True,
        num_devices=n_cores,
    )
    g_d = nc.dram_tensor("g", [R, C], F32, kind="ExternalInput")
    xf_d = nc.dram_tensor("xf", [1, R], F32, kind="ExternalInput")
    vsel_d = nc.dram_tensor("vsel", [1, R], F32, kind="ExternalInput")
    mneg_d = nc.dram_tensor("mneg", [KM, D], F32, kind="ExternalInput")
    t2_d = nc.dram_tensor("t2", [D, D], F32, kind="ExternalInput")
    t6_d = nc.dram_tensor("t6", [D, D], F32, kind="ExternalInput")
    t7_d = nc.dram_tensor("t7", [D, D], F32, kind="ExternalInput")
    t5c_d = nc.dram_tensor("t5c", [2 * D, 1], F32, kind="ExternalInput")
    out_d = nc.dram_tensor("out", [1, 1], F32, kind="ExternalOutput")
    ident_d = nc.inline_tensor(np.eye(128, dtype=np.float32), name="ident")

    rg = [list(range(n_cores))]

    with ExitStack() as ctx:
        tc = ctx.enter_context(tile.TileContext(nc))
        const = ctx.enter_context(tc.tile_pool(name="const", bufs=1))
        small = ctx.enter_context(tc.tile_pool(name="small", bufs=1))
        gp = ctx.enter_context(tc.tile_pool(name="gp", bufs=4))
        scr = ctx.enter_context(tc.tile_pool(name="scr", bufs=2))
        slp = ctx.enter_context(tc.tile_pool(name="sl", bufs=2))
        dram = ctx.enter_context(tc.tile_pool(name="dram", bufs=2, space="DRAM"))

        # ---- warm-up collective (absorbs first-CC init under phase 1's DMA);
        # same shape/kind as the real ones so the init covers them.
        dwi = dram.tile([1, D], F32, tag="dwi")
        dwo = dram.tile([n_cores, D], F32, tag="dwo")
        nc.gpsimd.collective_compute(
            "AllGather",
            ALU.bypass,
            replica_groups=rg,
            ins=[dwi[:].opt()],
            outs=[dwo[:].opt()],
        )

        # ---- constants / persistent tiles
        ident = const.tile([128, 128], F32)
        nc.scalar.dma_start(out=ident[:], in_=ident_d.ap())
        mneg = const.tile([KM, D], F32)
        nc.scalar.dma_start(out=mneg[:], in_=mneg_d.ap())
        # [A; A; B; B; theta1] rows at base partition 0, matching xs for u1
        # (sums first so the DVE copy into xs writes at partition offset 0)
        mneg_s = const.tile([KM - D, D], F32)
        nc.scalar.dma_start(out=mneg_s[0 : 2 * NCH, :], in_=mneg_d.ap()[D + 1 : KM, :])
        nc.scalar.dma_start(out=mneg_s[2 * NCH :, :], in_=mneg_d.ap()[D : D + 1, :])
        t2 = const.tile([D, D], F32)
        nc.scalar.dma_start(out=t2[:], in_=t2_d.ap())
        t6 = const.tile([D, D], F32)
        nc.scalar.dma_start(out=t6[:], in_=t6_d.ap())
        t7 = const.tile([D, D], F32)
        nc.scalar.dma_start(out=t7[:], in_=t7_d.ap())
        t5c = const.tile([2 * D, 1], F32)
        nc.scalar.dma_start(out=t5c[:], in_=t5c_d.ap())
        vsel = const.tile([1, R], F32)
        nc.scalar.dma_start(out=vsel[:], in_=vsel_d.ap())
        ones1d = const.tile([1, D], F32)
        nc.vector.memset(ones1d[:], 1.0)
        ones8 = const.tile([n_cores, 1], F32)
        nc.vector.memset(ones8[:], 1.0)

        # state tiles: [u (64 rows); xf; abs sums; plain sums]
        USa = small.tile([KM, R], F32)
        USb = small.tile([KM, R], F32)
        for us in (USa, USb):
            nc.scalar.dma_start(out=us[D : D + 1, :], in_=xf_d.ap())
        # compact [abs sums; plain sums; xf] tile: the u1 matmul reads this
        # directly so no SBUF->SBUF DMA sits on the phase-1 critical path
        xs = small.tile([1 + 2 * NCH, R], F32)
        nc.scalar.dma_start(out=xs[2 * NCH : 2 * NCH + 1, :], in_=xf_d.ap())

        # vsel replicated across the 64 partitions (for the final masked row)
        vrepS = small.tile([D, R], F32)
        with tc.tile_pool(name="psV", bufs=1, space="PSUM") as psV:
            vrep = psV.tile([D, R], F32, tag="vrep")
            for h in range(0, R, FB):
                he = min(R, h + FB)
                nc.tensor.matmul(
                    vrep[:, h:he],
                    lhsT=ones1d[:],
                    rhs=vsel[:, h:he],
                    start=True,
                    stop=True,
                )
            nc.vector.tensor_copy(vrepS[:], vrep[:])

        # ---- phase 1: stream graph; per-block pipeline down to u1
        # Each 128-row block b gets its own SPA column set.  The transpose +
        # u1 chain for block b is emitted two blocks late (after block b+2's
        # reduce/abs ops): engines execute their queues in order, and the
        # chain hops scalar -> tensor -> vector -> tensor -> scalar, so it
        # needs ~2 blocks of slack to never stall the streaming ops.  The
        # chain writes the transposed sums into xs (SBUF) and feeds the u1
        # matmul from there; the USa/USb state rows are filled by two
        # off-critical-path DMAs at the end.
        SPAs = [
            small.tile([128, 2 * NCH], F32, tag=f"spa{b}", name=f"spa{b}")
            for b in range(NB)
        ]
        slb = small.tile([D, NB], F32)  # per-block partial sums of u1
        with tc.tile_pool(name="psT", bufs=2, space="PSUM") as psT, tc.tile_pool(
            name="psU1", bufs=2, space="PSUM"
        ) as psU1:

            def u1_chain(b):
                lo, hi = b * 128, (b + 1) * 128
                tb = psT.tile([2 * NCH, 128], F32, tag="tb")
                nc.tensor.transpose(out=tb[:], in_=SPAs[b][:], identity=ident[:])
                nc.vector.tensor_copy(xs[0 : 2 * NCH, lo:hi], tb[:])
                ub = psU1.tile([D, 128], F32, tag="ub")
                nc.tensor.matmul(
                    ub[:],
                    lhsT=mneg_s[:],
                    rhs=xs[:, lo:hi],
                    start=True,
                    stop=True,
                )
                nc.scalar.activation(
                    out=USb[0:D, lo:hi],
                    in_=ub[:],
                    func=ACTF.Relu,
                    accum_out=slb[:, b : b + 1],
                )

            for b in range(NB):
                gt = gp.tile([128, C], F32, tag="gt")
                if (b == 0 or b == NB - 1) and NCH > 1:
                    # chunk-granular DMAs so the reduces pipeline with the
                    # transfer (first block: warm the pipe; last block: the
                    # serial tail is one chunk's work instead of a block's)
                    for k in range(NCH):
                        nc.sync.dma_start(
                            out=gt[:, k * CH : (k + 1) * CH],
                            in_=g_d.ap()[
                                b * 128 : (b + 1) * 128, k * CH : (k + 1) * CH
                            ],
                        )
                else:
                    nc.sync.dma_start(
                        out=gt[:], in_=g_d.ap()[b * 128 : (b + 1) * 128, :]
                    )
                for k in range(NCH):
                    nc.vector.reduce_sum(
                        out=SPAs[b][:, NCH + k : NCH + k + 1],
                        in_=gt[:, k * CH : (k + 1) * CH],
                        axis=AX.X,
                    )
                    st = scr.tile([128, CH], BF16, tag="st")
                    nc.scalar.activation(
                        out=st[:],
                        in_=gt[:, k * CH : (k + 1) * CH],
                        func=ACTF.Abs,
                        accum_out=SPAs[b][:, k : k + 1],
                    )
                if b >= 2:
                    u1_chain(b - 2)
            u1_chain(NB - 2)
            u1_chain(NB - 1)

        # ---- iterations 2..T (alternate state tiles)
        # Per-iteration global term: zrow = sl^T @ W  ([1, 64], W = t2 except
        # the last fold which uses t6 for the readout), AllGather -> [8, 64],
        # zs = AG^T @ ones8  ([64, 1]) is the activation bias (or q_in half).
        psPRE = ctx.enter_context(tc.tile_pool(name="psPRE", bufs=1, space="PSUM"))
        psS = ctx.enter_context(tc.tile_pool(name="psS", bufs=2, space="PSUM"))
        psC = ctx.enter_context(tc.tile_pool(name="psC", bufs=2, space="PSUM"))
        cur, nxt = USb, USa

        sl1 = slp.tile([D, 1], F32, tag="sl0", name="sl0")
        nc.vector.reduce_sum(out=sl1[:], in_=slb[:], axis=AX.X)
        W1 = t2 if T > 1 else t6
        zr_ps = psS.tile([1, D], F32, tag="zr")
        nc.tensor.matmul(zr_ps[:], lhsT=sl1[:], rhs=W1[:], start=True, stop=True)
        zrow = slp.tile([1, D], F32, tag="zrow")
        nc.scalar.copy(zrow[:], zr_ps[:])
        # fill the state tiles' sum rows for the iteration matmuls — gpsimd
        # (SWDGE) so it stays off the scalar/sync queues; completes during
        # the first AllGather, before the first pre-matmul needs it
        for us in (USb, USa):
            nc.gpsimd.dma_start(out=us[D + 1 : KM, :], in_=xs[0 : 2 * NCH, :])

        for t in range(1, T):
            cin = dram.tile([1, D], F32, tag="cin")
            cout = dram.tile([n_cores, D], F32, tag="cout")
            nc.scalar.dma_start(out=cin[:], in_=zrow[:])
            nc.gpsimd.collective_compute(
                "AllGather",
                ALU.bypass,
                replica_groups=rg,
                ins=[cin[:].opt()],
                outs=[cout[:].opt()],
            )
            ags = slp.tile([n_cores, D], F32, tag="ags")
            nc.scalar.dma_start(out=ags[:], in_=cout[:])

            pre = psPRE.tile([D, R], F32, tag="pre")
            for h in range(0, R, FB):
                he = min(R, h + FB)
                nc.tensor.matmul(
                    pre[:, h:he],
                    lhsT=mneg[:],
                    rhs=cur[:, h:he],
                    start=True,
                    stop=True,
                )
            zs_ps = psC.tile([D, 1], F32, tag="zs")
            nc.tensor.matmul(
                zs_ps[:], lhsT=ags[:], rhs=ones8[:], start=True, stop=True
            )
            zs = slp.tile([D, 1], F32, tag="zsl")
            nc.scalar.copy(zs[:], zs_ps[:])
            sl_nxt = slp.tile([D, 1], F32, tag="sln", name=f"sln{t}")
            nc.scalar.activation(
                out=nxt[0:D, :],
                in_=pre[:],
                func=ACTF.Relu,
                bias=zs[:, 0:1],
                accum_out=sl_nxt[:],
            )
            cur, nxt = nxt, cur
            Wt = t2 if t < T - 1 else t6
            zr_ps = psS.tile([1, D], F32, tag="zr")
            nc.tensor.matmul(
                zr_ps[:], lhsT=sl_nxt[:], rhs=Wt[:], start=True, stop=True
            )
            zrow = slp.tile([1, D], F32, tag="zrow")
            nc.scalar.copy(zrow[:], zr_ps[:])

        # ---- final readout
        # zrow now carries sl(u_T)^T @ t6; AllGather it once more.  v's row
        # lives on core 0 (v < R) and only core 0's output is read, so u[v]
        # needs no communication.
        with tc.tile_pool(name="psF", bufs=1, space="PSUM") as psF:
            uT = cur[0:D, :]
            uv = slp.tile([D, 1], F32, tag="uv")
            scrv = small.tile([D, R], F32)
            nc.vector.scalar_tensor_tensor(
                out=scrv[:],
                in0=uT,
                scalar=1.0,
                in1=vrepS[:],
                op0=ALU.mult,
                op1=ALU.mult,
                accum_out=uv[:],
            )
            cinf = dram.tile([1, D], F32, tag="cinf")
            coutf = dram.tile([n_cores, D], F32, tag="coutf")
            nc.scalar.dma_start(out=cinf[:], in_=zrow[:])
            nc.gpsimd.collective_compute(
                "AllGather",
                ALU.bypass,
                replica_groups=rg,
                ins=[cinf[:].opt()],
                outs=[coutf[:].opt()],
            )
            agf = slp.tile([n_cores, D], F32, tag="agf")
            nc.scalar.dma_start(out=agf[:], in_=coutf[:])

            q = psF.tile([2 * D, 1], F32, tag="q")
            nc.tensor.matmul(
                q[0:D, :], lhsT=agf[:], rhs=ones8[:], start=True, stop=True
            )
            nc.tensor.matmul(
                q[D : 2 * D, :], lhsT=t7[:], rhs=uv[:], start=True, stop=True
            )
            rq = small.tile([2 * D, 1], F32)
            nc.scalar.activation(out=rq[:], in_=q[:], func=ACTF.Relu)
            res = psF.tile([1, 1], F32, tag="res")
            nc.tensor.matmul(
                res[:], lhsT=rq[:], rhs=t5c[:], start=True, stop=True
            )
            ress = small.tile([1, 1], F32)
            nc.scalar.copy(ress[:], res[:])
            nc.scalar.dma_start(out=out_d.ap(), in_=ress[:])

    nc.compile()
    return nc


def get_program(R: int, C: int, D: int, T: int, n_cores: int = N_CORES):
    key = (R, C, D, T, n_cores)
    if key not in _program_cache:
        _program_cache[key] = build_program(R, C, D, T, n_cores)
    return _program_cache[key]


def make_in_maps(graph, x, theta1, theta2, theta3, theta4, theta5, theta6, theta7, v,
                 n_cores: int = N_CORES):
    """Host-side sharding + tiny theta preprocessing."""
    N = graph.shape[0]
    D = theta1.shape[1]
    R = N // n_cores
    _, NCH = _chunking(N)
    f32 = np.float32

    t4 = np.asarray(theta4, f32)[0]
    t3 = np.asarray(theta3, f32)
    A = 0.5 * (np.abs(t4) @ t3)
    B = 0.5 * (t4 @ t3)
    t2 = np.ascontiguousarray(np.asarray(theta2, f32))
    mneg = np.ascontiguousarray(
        np.concatenate(
            [-t2, np.asarray(theta1, f32)]
            + [A[None, :]] * NCH
            + [B[None, :]] * NCH,
            axis=0,
        ).astype(f32)
    )  # (D + 1 + 2*NCH, D)
    t5c = np.ascontiguousarray(np.asarray(theta5, f32).reshape(2 * D, 1))
    t6 = np.ascontiguousarray(np.asarray(theta6, f32))
    t7 = np.ascontiguousarray(np.asarray(theta7, f32))

    xf = np.asarray(x).astype(f32)
    vsel_full = np.zeros(N, f32)
    vsel_full[int(v)] = 1.0

    in_maps = []
    for i in range(n_cores):
        sl = slice(i * R, (i + 1) * R)
        in_maps.append(
            {
                "g": np.ascontiguousarray(np.asarray(graph, f32)[sl]),
                "xf": np.ascontiguousarray(xf[sl].reshape(1, R)),
                "vsel": np.ascontiguousarray(vsel_full[sl].reshape(1, R)),
                "mneg": mneg,
                "t2": t2,
                "t6": t6,
                "t7": t7,
                "t5c": t5c,
            }
        )
    return in_maps


def run(inputs: dict, trace: bool = False):
    """Run the distributed kernel on hardware; returns (output, BassKernelResults)."""
    graph = np.asarray(inputs["graph"])
    N = graph.shape[0]
    D = inputs["theta1"].shape[1]
    T = int(inputs["T"])
    R = N // N_CORES
    # the final readout takes u[v] from core 0 locally (no collective)
    assert int(inputs["v"]) < R, "v must live in core 0's row shard"

    nc = get_program(R, N, D, T, N_CORES)
    in_maps = make_in_maps(
        graph,
        inputs["x"],
        inputs["theta1"],
        inputs["theta2"],
        inputs["theta3"],
        inputs["theta4"],
        inputs["theta5"],
        inputs["theta6"],
        inputs["theta7"],
        inputs["v"],
        N_CORES,
    )
    res = run_bass_kernel_spmd(
        nc, in_maps, core_ids=list(range(N_CORES)), trace=trace
    )
    out = np.asarray(res.results[0]["out"], np.float32).reshape(1)
    return out, res


def kernel(**inputs) -> np.ndarray:
    out, _ = run(inputs, trace=False)
    return out


# revision 17
# speedup vs baseline: 1.0226x; 1.0226x over previous
"""Distributed Trainium2 (Bass/Tile) kernel for the DQN-style GNN message-passing
module.

Full-input contract: ``kernel(**inputs)`` takes the unsharded inputs exactly as
produced by ``setup_inputs()`` and returns the full output (shape ``(1,)``).

Sharding strategy (1D node partition, per the hint):
  - graph [N, N] row-sharded across 8 cores -> [N/8, N] per core
  - x / u row-sharded; thetas replicated
  - per-iteration global reduction: the local per-core sum sl = u.sum(rows) is
    folded through theta2 BEFORE communicating (AllReduce is linear), giving a
    [1, 64] row that is AllGathered (floor ~4.6us vs AllReduce ~9.7us) and
    combined on-chip with a ones-vector matmul.  The [1, 64] single-partition
    staging layout makes the HBM bounce DMA a single 256B descriptor instead
    of 64x4B (whose completion receipts cost ~10us+).
  - final readout: only core 0's output is read, and v's row lives on core 0,
    so the last collective is the same [1,64] AllGather with theta6 folded in.

Math (mirrors the reference's exact relu identity):
  c[r, :] = s_abs[r] * A + s_sum[r] * B
    with s_abs[r] = sum_j |g[r, j]|, s_sum[r] = sum_j g[r, j],
         A = 0.5 * |theta4| @ theta3, B = 0.5 * theta4 @ theta3
  a[r, :] = xf[r] * theta1

  Per-core state tile US [64 + 1 + 2*NCH, R] stacks u^T over per-row scalars
  [xf; abs chunk sums; plain chunk sums].  With the host-built stationary
  M = [-theta2; theta1; A (x NCH); B (x NCH)], a single matmul computes
    pre^T = M^T @ US = (a + c - u @ theta2)^T
  and the iteration is  u' = ReluAct(pre, bias = zs)  where zs [64,1] is the
  ones-combine of the AllGathered t2-folded rows.
"""

from contextlib import ExitStack

import numpy as np

import concourse.bass as bass
import concourse.tile as tile
from concourse import bacc, mybir
from concourse.bass_utils import run_bass_kernel_spmd

F32 = mybir.dt.float32
BF16 = mybir.dt.bfloat16
AX = mybir.AxisListType
ALU = mybir.AluOpType
ACTF = mybir.ActivationFunctionType

N_CORES = 8
DIM = 64

_program_cache: dict = {}


def _chunking(C: int):
    CH = 2048 if C >= 2048 else C
    return CH, C // CH


def build_program(R: int, C: int, D: int, T: int, n_cores: int = N_CORES):
    """Build + compile the per-core SPMD Bass program.

    R: local rows (N / n_cores), C: row length (N), D: dim, T: iterations.
    """
    assert R % 128 == 0 and D == 64
    NB = R // 128
    CH, NCH = _chunking(C)
    KM = D + 1 + 2 * NCH  # stationary contraction size
    FB = 512  # matmul free-dim chunk (one PSUM bank of f32)

    nc = bacc.Bacc(
        "TRN2",
        target_bir_lowering=False,
        debug=False,
        enable_asserts=True,
        num_devices=n_cores,
    )
    g_d = nc.dram_tensor("g", [R, C], F32, kind="ExternalInput")
    xf_d = nc.dram_tensor("xf", [1, R], F32, kind="ExternalInput")
    vsel_d = nc.dram_tensor("vsel", [1, R], F32, kind="ExternalInput")
    mneg_d = nc.dram_tensor("mneg", [KM, D], F32, kind="ExternalInput")
    t2_d = nc.dram_tensor("t2", [D, D], F32, kind="ExternalInput")
    t6_d = nc.dram_tensor("t6", [D, D], F32, kind="ExternalInput")
    t7_d = nc.dram_tensor("t7", [D, D], F32, kind="ExternalInput")
    t5c_d = nc.dram_tensor("t5c", [2 * D, 1], F32, kind="ExternalInput")
    out_d = nc.dram_tensor("out", [1, 1], F32, kind="ExternalOutput")
    ident_d = nc.inline_tensor(np.eye(128, dtype=np.float32), name="ident")

    rg = [list(range(n_cores))]

    with ExitStack() as ctx:
        tc = ctx.enter_context(tile.TileContext(nc))
        const = ctx.enter_context(tc.tile_pool(name="const", bufs=1))
        small = ctx.enter_context(tc.tile_pool(name="small", bufs=1))
        gp = ctx.enter_context(tc.tile_pool(name="gp", bufs=4))
        scr = ctx.enter_context(tc.tile_pool(name="scr", bufs=2))
        slp = ctx.enter_context(tc.tile_pool(name="sl", bufs=2))
        dram = ctx.enter_context(tc.tile_pool(name="dram", bufs=2, space="DRAM"))

        # ---- warm-up collective (absorbs first-CC init under phase 1's DMA);
        # same shape/kind as the real ones so the init covers them.
        dwi = dram.tile([1, D], F32, tag="dwi")
        dwo = dram.tile([n_cores, D], F32, tag="dwo")
        nc.gpsimd.collective_compute(
            "AllGather",
            ALU.bypass,
            replica_groups=rg,
            ins=[dwi[:].opt()],
            outs=[dwo[:].opt()],
        )

        # ---- constants / persistent tiles
        ident = const.tile([128, 128], F32)
        nc.scalar.dma_start(out=ident[:], in_=ident_d.ap())
        mneg = const.tile([KM, D], F32)
        nc.scalar.dma_start(out=mneg[:], in_=mneg_d.ap())
        # [A; A; B; B; theta1] rows at base partition 0, matching xs for u1
        # (sums first so the DVE copy into xs writes at partition offset 0)
        mneg_s = const.tile([KM - D, D], F32)
        nc.scalar.dma_start(out=mneg_s[0 : 2 * NCH, :], in_=mneg_d.ap()[D + 1 : KM, :])
        nc.scalar.dma_start(out=mneg_s[2 * NCH :, :], in_=mneg_d.ap()[D : D + 1, :])
        t2 = const.tile([D, D], F32)
        nc.scalar.dma_start(out=t2[:], in_=t2_d.ap())
        t6 = const.tile([D, D], F32)
        nc.scalar.dma_start(out=t6[:], in_=t6_d.ap())
        t7 = const.tile([D, D], F32)
        nc.scalar.dma_start(out=t7[:], in_=t7_d.ap())
        t5c = const.tile([2 * D, 1], F32)
        nc.scalar.dma_start(out=t5c[:], in_=t5c_d.ap())
        vsel = const.tile([1, R], F32)
        nc.scalar.dma_start(out=vsel[:], in_=vsel_d.ap())
        ones1d = const.tile([1, D], F32)
        nc.vector.memset(ones1d[:], 1.0)
        ones8 = const.tile([n_cores, 1], F32)
        nc.vector.memset(ones8[:], 1.0)

        # state tiles: [u (64 rows); xf; abs sums; plain sums]
        USa = small.tile([KM, R], F32)
        USb = small.tile([KM, R], F32)
        for us in (USa, USb):
            nc.scalar.dma_start(out=us[D : D + 1, :], in_=xf_d.ap())
        # compact [abs sums; plain sums; xf] tile: the u1 matmul reads this
        # directly so no SBUF->SBUF DMA sits on the phase-1 critical path
        xs = small.tile([1 + 2 * NCH, R], F32)
        nc.scalar.dma_start(out=xs[2 * NCH : 2 * NCH + 1, :], in_=xf_d.ap())

        # vsel replicated across the 64 partitions (for the final masked row)
        vrepS = small.tile([D, R], F32)
        with tc.tile_pool(name="psV", bufs=1, space="PSUM") as psV:
            vrep = psV.tile([D, R], F32, tag="vrep")
            for h in range(0, R, FB):
                he = min(R, h + FB)
                nc.tensor.matmul(
                    vrep[:, h:he],
                    lhsT=ones1d[:],
                    rhs=vsel[:, h:he],
                    start=True,
                    stop=True,
                )
            nc.vector.tensor_copy(vrepS[:], vrep[:])

        # ---- phase 1: stream graph; per-block pipeline down to u1
        # Each 128-row block b gets its own SPA column set.  The transpose +
        # u1 chain for block b is emitted two blocks late (after block b+2's
        # reduce/abs ops): engines execute their queues in order, and the
        # chain hops scalar -> tensor -> vector -> tensor -> scalar, so it
        # needs ~2 blocks of slack to never stall the streaming ops.  The
        # chain writes the transposed sums into xs (SBUF) and feeds the u1
        # matmul from there; the USa/USb state rows are filled by two
        # off-critical-path DMAs at the end.
        SPAs = [
            small.tile([128, 2 * NCH], F32, tag=f"spa{b}", name=f"spa{b}")
            for b in range(NB)
        ]
        slb = small.tile([D, NB], F32)  # per-block partial sums of u1
        with tc.tile_pool(name="psT", bufs=2, space="PSUM") as psT, tc.tile_pool(
            name="psU1", bufs=2, space="PSUM"
        ) as psU1:

            def u1_chain(b):
                lo, hi = b * 128, (b + 1) * 128
                tb = psT.tile([2 * NCH, 128], F32, tag="tb")
                nc.tensor.transpose(out=tb[:], in_=SPAs[b][:], identity=ident[:])
                nc.vector.tensor_copy(xs[0 : 2 * NCH, lo:hi], tb[:])
                ub = psU1.tile([D, 128], F32, tag="ub")
                nc.tensor.matmul(
                    ub[:],
                    lhsT=mneg_s[:],
                    rhs=xs[:, lo:hi],
                    start=True,
                    stop=True,
                )
                nc.scalar.activation(
                    out=USb[0:D, lo:hi],
                    in_=ub[:],
                    func=ACTF.Relu,
                    accum_out=slb[:, b : b + 1],
                )

            for b in range(NB):
                gt = gp.tile([128, C], F32, tag="gt")
                # chunk-granular DMAs: the reduces start as each 1MB chunk
                # lands instead of waiting for the whole 4MB block, so the
                # engines trail the stream by ~one chunk, not one block
                for k in range(NCH):
                    nc.sync.dma_start(
                        out=gt[:, k * CH : (k + 1) * CH],
                        in_=g_d.ap()[
                            b * 128 : (b + 1) * 128, k * CH : (k + 1) * CH
                        ],
                    )
                for k in range(NCH):
                    nc.vector.reduce_sum(
                        out=SPAs[b][:, NCH + k : NCH + k + 1],
                        in_=gt[:, k * CH : (k + 1) * CH],
                        axis=AX.X,
                    )
                    st = scr.tile([128, CH], BF16, tag="st")
                    nc.scalar.activation(
                        out=st[:],
                        in_=gt[:, k * CH : (k + 1) * CH],
                        func=ACTF.Abs,
                        accum_out=SPAs[b][:, k : k + 1],
                    )
                if b >= 2:
                    u1_chain(b - 2)
            u1_chain(NB - 2)
            u1_chain(NB - 1)

        # ---- iterations 2..T (alternate state tiles)
        # Per-iteration global term: zrow = sl^T @ W  ([1, 64], W = t2 except
        # the last fold which uses t6 for the readout), AllGather -> [8, 64],
        # zs = AG^T @ ones8  ([64, 1]) is the activation bias (or q_in half).
        psPRE = ctx.enter_context(tc.tile_pool(name="psPRE", bufs=1, space="PSUM"))
        psS = ctx.enter_context(tc.tile_pool(name="psS", bufs=2, space="PSUM"))
        psC = ctx.enter_context(tc.tile_pool(name="psC", bufs=2, space="PSUM"))
        cur, nxt = USb, USa

        sl1 = slp.tile([D, 1], F32, tag="sl0", name="sl0")
        nc.vector.reduce_sum(out=sl1[:], in_=slb[:], axis=AX.X)
        W1 = t2 if T > 1 else t6
        zr_ps = psS.tile([1, D], F32, tag="zr")
        nc.tensor.matmul(zr_ps[:], lhsT=sl1[:], rhs=W1[:], start=True, stop=True)
        zrow = slp.tile([1, D], F32, tag="zrow")
        nc.scalar.copy(zrow[:], zr_ps[:])
        # fill the state tiles' sum rows for the iteration matmuls — gpsimd
        # (SWDGE) so it stays off the scalar/sync queues; completes during
        # the first AllGather, before the first pre-matmul needs it
        for us in (USb, USa):
            nc.gpsimd.dma_start(out=us[D + 1 : KM, :], in_=xs[0 : 2 * NCH, :])

        for t in range(1, T):
            cin = dram.tile([1, D], F32, tag="cin")
            cout = dram.tile([n_cores, D], F32, tag="cout")
            nc.scalar.dma_start(out=cin[:], in_=zrow[:])
            nc.gpsimd.collective_compute(
                "AllGather",
                ALU.bypass,
                replica_groups=rg,
                ins=[cin[:].opt()],
                outs=[cout[:].opt()],
            )
            ags = slp.tile([n_cores, D], F32, tag="ags")
            nc.scalar.dma_start(out=ags[:], in_=cout[:])

            pre = psPRE.tile([D, R], F32, tag="pre")
            for h in range(0, R, FB):
                he = min(R, h + FB)
                nc.tensor.matmul(
                    pre[:, h:he],
                    lhsT=mneg[:],
                    rhs=cur[:, h:he],
                    start=True,
                    stop=True,
                )
            zs_ps = psC.tile([D, 1], F32, tag="zs")
            nc.tensor.matmul(
                zs_ps[:], lhsT=ags[:], rhs=ones8[:], start=True, stop=True
            )
            zs = slp.tile([D, 1], F32, tag="zsl")
            nc.scalar.copy(zs[:], zs_ps[:])
            sl_nxt = slp.tile([D, 1], F32, tag="sln", name=f"sln{t}")
            nc.scalar.activation(
                out=nxt[0:D, :],
                in_=pre[:],
                func=ACTF.Relu,
                bias=zs[:, 0:1],
                accum_out=sl_nxt[:],
            )
            cur, nxt = nxt, cur
            Wt = t2 if t < T - 1 else t6
            zr_ps = psS.tile([1, D], F32, tag="zr")
            nc.tensor.matmul(
                zr_ps[:], lhsT=sl_nxt[:], rhs=Wt[:], start=True, stop=True
            )
            zrow = slp.tile([1, D], F32, tag="zrow")
            nc.scalar.copy(zrow[:], zr_ps[:])

        # ---- final readout
        # zrow now carries sl(u_T)^T @ t6; AllGather it once more.  v's row
        # lives on core 0 (v < R) and only core 0's output is read, so u[v]
        # needs no communication.
        with tc.tile_pool(name="psF", bufs=1, space="PSUM") as psF:
            uT = cur[0:D, :]
            uv = slp.tile([D, 1], F32, tag="uv")
            scrv = small.tile([D, R], F32)
            nc.vector.scalar_tensor_tensor(
                out=scrv[:],
                in0=uT,
                scalar=1.0,
                in1=vrepS[:],
                op0=ALU.mult,
                op1=ALU.mult,
                accum_out=uv[:],
            )
            cinf = dram.tile([1, D], F32, tag="cinf")
            coutf = dram.tile([n_cores, D], F32, tag="coutf")
            nc.scalar.dma_start(out=cinf[:], in_=zrow[:])
            nc.gpsimd.collective_compute(
                "AllGather",
                ALU.bypass,
                replica_groups=rg,
                ins=[cinf[:].opt()],
                outs=[coutf[:].opt()],
            )
            agf = slp.tile([n_cores, D], F32, tag="agf")
            nc.scalar.dma_start(out=agf[:], in_=coutf[:])

            q = psF.tile([2 * D, 1], F32, tag="q")
            nc.tensor.matmul(
                q[0:D, :], lhsT=agf[:], rhs=ones8[:], start=True, stop=True
            )
            nc.tensor.matmul(
                q[D : 2 * D, :], lhsT=t7[:], rhs=uv[:], start=True, stop=True
            )
            rq = small.tile([2 * D, 1], F32)
            nc.scalar.activation(out=rq[:], in_=q[:], func=ACTF.Relu)
            res = psF.tile([1, 1], F32, tag="res")
            nc.tensor.matmul(
                res[:], lhsT=rq[:], rhs=t5c[:], start=True, stop=True
            )
            ress = small.tile([1, 1], F32)
            nc.scalar.copy(ress[:], res[:])
            nc.scalar.dma_start(out=out_d.ap(), in_=ress[:])

    nc.compile()
    return nc


def get_program(R: int, C: int, D: int, T: int, n_cores: int = N_CORES):
    key = (R, C, D, T, n_cores)
    if key not in _program_cache:
        _program_cache[key] = build_program(R, C, D, T, n_cores)
    return _program_cache[key]


def make_in_maps(graph, x, theta1, theta2, theta3, theta4, theta5, theta6, theta7, v,
                 n_cores: int = N_CORES):
    """Host-side sharding + tiny theta preprocessing."""
    N = graph.shape[0]
    D = theta1.shape[1]
    R = N // n_cores
    _, NCH = _chunking(N)
    f32 = np.float32

    t4 = np.asarray(theta4, f32)[0]
    t3 = np.asarray(theta3, f32)
    A = 0.5 * (np.abs(t4) @ t3)
    B = 0.5 * (t4 @ t3)
    t2 = np.ascontiguousarray(np.asarray(theta2, f32))
    mneg = np.ascontiguousarray(
        np.concatenate(
            [-t2, np.asarray(theta1, f32)]
            + [A[None, :]] * NCH
            + [B[None, :]] * NCH,
            axis=0,
        ).astype(f32)
    )  # (D + 1 + 2*NCH, D)
    t5c = np.ascontiguousarray(np.asarray(theta5, f32).reshape(2 * D, 1))
    t6 = np.ascontiguousarray(np.asarray(theta6, f32))
    t7 = np.ascontiguousarray(np.asarray(theta7, f32))

    xf = np.asarray(x).astype(f32)
    vsel_full = np.zeros(N, f32)
    vsel_full[int(v)] = 1.0

    in_maps = []
    for i in range(n_cores):
        sl = slice(i * R, (i + 1) * R)
        in_maps.append(
            {
                "g": np.ascontiguousarray(np.asarray(graph, f32)[sl]),
                "xf": np.ascontiguousarray(xf[sl].reshape(1, R)),
                "vsel": np.ascontiguousarray(vsel_full[sl].reshape(1, R)),
                "mneg": mneg,
                "t2": t2,
                "t6": t6,
                "t7": t7,
                "t5c": t5c,
            }
        )
    return in_maps


def run(inputs: dict, trace: bool = False):
    """Run the distributed kernel on hardware; returns (output, BassKernelResults)."""
    graph = np.asarray(inputs["graph"])
    N = graph.shape[0]
    D = inputs["theta1"].shape[1]
    T = int(inputs["T"])
    R = N // N_CORES
    # the final readout takes u[v] from core 0 locally (no collective)
    assert int(inputs["v"]) < R, "v must live in core 0's row shard"

    nc = get_program(R, N, D, T, N_CORES)
    in_maps = make_in_maps(
        graph,
        inputs["x"],
        inputs["theta1"],
        inputs["theta2"],
        inputs["theta3"],
        inputs["theta4"],
        inputs["theta5"],
        inputs["theta6"],
        inputs["theta7"],
        inputs["v"],
        N_CORES,
    )
    res = run_bass_kernel_spmd(
        nc, in_maps, core_ids=list(range(N_CORES)), trace=trace
    )
    out = np.asarray(res.results[0]["out"], np.float32).reshape(1)
    return out, res


def kernel(**inputs) -> np.ndarray:
    out, _ = run(inputs, trace=False)
    return out
